# revision 41
# baseline (speedup 1.0000x reference)
"""Coupled-attention module as a distributed Bass/Tile kernel on 8 TRN2 cores.

Math notes (exact algebra, not approximations):
- The differential-attention scores are constant along the softmax axis, so
  softmax yields exactly uniform 1/S weights: diff_vector collapses to the
  per-batch mean of (y @ dv_w + dv_b), broadcast over sequence. dq/dk are dead.
- Sharding: rows of the flattened (B*S, H) activations, 256 per core; cores
  0-3 own batch 0, 4-7 batch 1. Each core redundantly computes full-batch K/V
  (cheaper than any reshard collective at this scale).
- All activations live channel-major [C, rows] on chip, so weights feed the
  PE as natural [K, M] lhsT tiles, and the two sequence-axis softmaxes in the
  gating network reduce along the free dim. Their denominators are summed
  across the 4-core batch group with tiny AllReduces.
- Compute in bf16 with fp32 accumulation (all GEMMs), exp/tanh on ACT.
- Attention softmax normalization is deferred: per head the unnormalized
  PV and the Z row are copied out, then ONE batched reciprocal [12, 256]
  plus a selector-matmul broadcast rescales all heads (the per-head DVE
  reciprocal at [1, 256] costs the same as the whole batch).
- Sigmoids are computed as 0.5*(1+tanh(x/2)) to stay in the exp/tanh ACT
  table set (avoids a ~1.5us ACT table switch).
- no_sync_barrier after each AllReduce input keeps the filler GEMMs from
  being consumed early, so they land inside the collective's window.
"""

import numpy as np
import ml_dtypes

import concourse.bass as bass
import concourse.mybir as mybir
import concourse.tile as tile
from concourse import bacc
from concourse.bass_utils import run_bass_kernel_spmd

B, S, H = 2, 1024, 768
NH, DH = 12, 64
P = 128
RV = 256            # rows per core
KC = H // P         # 6 channel chunks
JC = S // P         # 8 sequence chunks
GROUPS = [[0, 1, 2, 3], [4, 5, 6, 7]]
SCALE = 1.0 / 8.0   # 1/sqrt(DH)

bf16 = mybir.dt.bfloat16
f32 = mybir.dt.float32
AF = mybir.ActivationFunctionType
ALU = mybir.AluOpType
nbf16 = ml_dtypes.bfloat16

W768 = ["vq_w", "vk_w", "vv_w", "dv_w", "WD_w", "van_fc_w", "WV_w", "diff_fc_w",
        "diff_fus_w", "van_fus_w", "nf_w", "final_w"]
W1536 = ["d_theta_w", "v_gamma_w", "diff_out_w", "van_out_w"]
BIAS = ["vq_b", "vk_b", "dv_b", "van_fc_b", "d_theta_b", "diff_fc_b",
        "v_gamma_b", "diff_out_b", "van_out_b", "diff_fus_b", "van_fus_b",
        "nf_b", "final_b"]


def build(has_vvb: bool):
    nc = bacc.Bacc(None, target_bir_lowering=False, debug=False, num_devices=8)

    xT_d = nc.dram_tensor("xT", [H, RV], bf16, kind="ExternalInput")
    yT_d = nc.dram_tensor("yT", [H, S], bf16, kind="ExternalInput")
    wd = {}
    for w in W768:
        wd[w] = nc.dram_tensor(w, [H, H], bf16, kind="ExternalInput")
    for w in W1536:
        wd[w] = nc.dram_tensor(w, [2 * H, H], bf16, kind="ExternalInput")
    wd["gate_w"] = nc.dram_tensor("gate_w", [2 * H, 1], bf16, kind="ExternalInput")
    wd["nf_out_w"] = nc.dram_tensor("nf_out_w", [2 * H, 1], bf16, kind="ExternalInput")
    bd = {}
    for b in BIAS:
        bd[b] = nc.dram_tensor(b, [H], f32, kind="ExternalInput")
    if has_vvb:
        bd["vv_b"] = nc.dram_tensor("vv_b", [H], f32, kind="ExternalInput")
    sel_d = nc.dram_tensor("selM", [P, 2 * P], bf16, kind="ExternalInput")
    out_d = nc.dram_tensor("outT", [H, RV], f32, kind="ExternalOutput")

    with tile.TileContext(nc, num_cores=8) as tc:
        with (
            tc.tile_pool(name="wpool", bufs=6) as wp,
            tc.tile_pool(name="wsmall", bufs=2) as wsp,
            tc.tile_pool(name="acts", bufs=1) as ap,
            tc.tile_pool(name="loop", bufs=2) as lp,
            tc.tile_pool(name="psum", bufs=8, space="PSUM") as pp,
            tc.tile_pool(name="dram", bufs=4, space="DRAM") as dp,
        ):
            # PSUM budget: tag "ps" = 6 x 1-bank [128, 512] f32 slots shared
            # by every projection/score/gating matmul; tag "pv" = 2 x 1-bank
            # [65, 2, 256] accumulators. 6 + 2 = 8 banks exactly.
            def psum(shape, name):
                return pp.tile(shape, f32, name=name, tag="ps", bufs=6)

            # -------- group reduce via ncfw AllReduce. The accum ACTs write
            # the partial into part[r]; the reduced value is read back from
            # DRAM. A dummy warmup AllReduce early in the kernel absorbs the
            # ~12us first-collective ncfw spin-up observed on every trace.
            parts = [ap.tile([P, KC], f32, name=f"part{r}") for r in range(2)]

            def remote_reduce_start(r):
                ci = dp.tile([P, KC], f32, name=f"ci{r}")
                co = dp.tile([P, KC], f32, name=f"co{r}")
                nc.sync.dma_start(ci[:], parts[r][:])
                nc.gpsimd.collective_compute(
                    "AllReduce", ALU.add, replica_groups=GROUPS,
                    ins=[ci[:]], outs=[co[:]])
                return co

            def remote_reduce_finish(co, name):
                z = ap.tile([P, KC], f32, name=f"zz_{name}")
                nc.sync.dma_start(z[:], co[:])
                return z

            def wtile(name, half=None):
                t = wp.tile([P, KC, H], bf16, name=f"w_{name}_{half}", tag="w")
                src = wd[name]
                if half is not None:
                    src = src[half * H:(half + 1) * H, :]
                src = src.rearrange("(kc p) n -> kc p n", p=P)
                for kc in range(KC):
                    nc.sync.dma_start(t[:, kc, :], src[kc])
                return t

            def btile(name):
                t = ap.tile([P, KC], f32, name=f"b_{name}")
                nc.sync.dma_start(t[:], bd[name].rearrange("(c p) -> p c", p=P))
                return t

            # K-side inputs stream first: K proj is the startup long pole.
            b_vk = btile("vk_b")
            b_dv = btile("dv_b")
            yT = ap.tile([P, KC, S], bf16, name="yT")
            for kc in range(KC):
                nc.sync.dma_start(yT[:, kc, :], yT_d.rearrange(
                    "(kc p) n -> kc p n", p=P)[kc])
            w_vk0 = wtile("vk_w")

            b_vq = btile("vq_b")
            xT = ap.tile([P, KC, RV], bf16, name="xT")
            for kc in range(KC):
                nc.sync.dma_start(xT[:, kc, :], xT_d.rearrange(
                    "(kc p) n -> kc p n", p=P)[kc])
            w_vq = wtile("vq_w")
            qT = ap.tile([P, KC, RV], bf16, name="qT")
            for mc in range(KC):
                ps = psum([P, RV], f"qps{mc}")
                for kc in range(KC):
                    nc.tensor.matmul(ps[:], w_vq[:, kc, mc * P:(mc + 1) * P],
                                     xT[:, kc, :],
                                     start=(kc == 0), stop=(kc == KC - 1))
                nc.scalar.activation(qT[:, mc, :], ps[:], AF.Identity,
                                     bias=b_vq[:, mc:mc + 1])

            ones128 = ap.tile([1, P], f32, name="ones128")
            nc.vector.memset(ones128[:], 1.0)

            # dummy warmup collective: absorbs the ~12us ncfw first-op
            # spin-up while the PE is busy with projections
            wu = ap.tile([1, 1], f32, name="wu")
            nc.vector.memset(wu[:], 0.0)
            wci = dp.tile([1, 1], f32, name="wci")
            wco = dp.tile([1, 1], f32, name="wco")
            nc.sync.dma_start(wci[:], wu[:])
            nc.gpsimd.collective_compute(
                "AllReduce", ALU.add, replica_groups=GROUPS,
                ins=[wci[:]], outs=[wco[:]])

            # selector for broadcasting invZ rows (at partitions 0/32/64/96)
            # onto the two channel chunks each zbig tile covers
            sel = ap.tile([P, 2, P], bf16, name="sel")
            nc.sync.dma_start(sel[:], sel_d.rearrange("p (c q) -> p c q", q=P))

            # ---------------- K / V projections (emitted interleaved with
            # the attention pairs below so PE stays dense through the
            # ACT-bound exp phases; HAM stays warm) -----------------------
            w_vk = w_vk0
            kT = ap.tile([P, KC, S], bf16, name="kT")

            def kproj(mc):
                for nh in range(2):
                    ps = psum([P, 512], f"kps{mc}_{nh}")
                    for kc in range(KC):
                        nc.tensor.matmul(
                            ps[:], w_vk[:, kc, mc * P:(mc + 1) * P],
                            yT[:, kc, nh * 512:(nh + 1) * 512],
                            start=(kc == 0), stop=(kc == KC - 1))
                    nc.scalar.activation(kT[:, mc, nh * 512:(nh + 1) * 512], ps[:],
                                         AF.Identity, bias=b_vk[:, mc:mc + 1])

            w_vv = wtile("vv_w")
            v_aug = ap.tile([P, JC, NH, DH + 1], bf16, name="v_aug")
            nc.vector.memset(v_aug[:, :, :, DH:DH + 1], 1.0)

            def vproj(cg):
                for jc in range(JC):
                    ps = psum([P, 384], f"vps{jc}_{cg}")
                    for kc in range(KC):
                        nc.tensor.matmul(
                            ps[:], yT[:, kc, jc * P:(jc + 1) * P],
                            w_vv[:, kc, cg * 384:(cg + 1) * 384],
                            start=(kc == 0), stop=(kc == KC - 1))
                    nc.vector.tensor_copy(
                        v_aug[:, jc, cg * 6:(cg + 1) * 6, 0:DH],
                        ps[:].rearrange("p (h d) -> p h d", d=DH))

            for mc in range(3):
                kproj(mc)

            # ---------------- diff-branch constants (per batch) -------------
            # m = mean_s(y) @ dv_w + dv_b ; theta1 = tanh(m @ WD_w)
            # bias1 = theta1 @ d_theta_w[:H] + d_theta_b
            # bias2 = m @ diff_out_w[:H] + diff_out_b
            yb = ap.tile([P, KC], f32, name="yb")
            ybt = ap.tile([P, KC], bf16, name="ybt")
            for kc in range(KC):
                nc.vector.tensor_reduce(yb[:, kc:kc + 1], yT[:, kc, :],
                                        axis=mybir.AxisListType.X, op=ALU.add)
            nc.vector.tensor_scalar_mul(ybt[:], yb[:], 1.0 / S)

            def vec_chain(w_t, rhs_t, func, bias_t, out_dt, name):
                out = ap.tile([P, KC], out_dt, name=name)
                for mc in range(KC):
                    ps = psum([P, 1], f"{name}ps{mc}")
                    for kc in range(KC):
                        nc.tensor.matmul(ps[:], w_t[:, kc, mc * P:(mc + 1) * P],
                                         rhs_t[:, kc:kc + 1],
                                         start=(kc == 0), stop=(kc == KC - 1))
                    nc.scalar.activation(out[:, mc:mc + 1], ps[:], func,
                                         bias=(bias_t[:, mc:mc + 1]
                                               if bias_t is not None else 0.0))
                return out

            w_dv = wtile("dv_w")
            m32 = vec_chain(w_dv, ybt, AF.Identity, b_dv, f32, "m32")
            mbf = ap.tile([P, KC], bf16, name="mbf")
            nc.vector.tensor_copy(mbf[:], m32[:])
            w_WD = wtile("WD_w")
            th1 = vec_chain(w_WD, mbf, AF.Tanh, None, bf16, "th1")
            w_dth0 = wtile("d_theta_w", half=0)
            b_dth = btile("d_theta_b")
            bias1 = vec_chain(w_dth0, th1, AF.Identity, b_dth, f32, "bias1")
            w_dout0 = wtile("diff_out_w", half=0)
            b_dout = btile("diff_out_b")
            bias2 = vec_chain(w_dout0, mbf, AF.Identity, b_dout, f32, "bias2")

            # ---------------- attention (12 heads, 256 own queries) ---------
            # Unnormalized PV + Z rows per head; normalization batched in two
            # head groups (0-7, 8-11) so vanT chunks 0-3 free up early.
            if has_vvb:
                b_vv = btile("vv_b")
            van_un = ap.tile([P, KC, RV], bf16, name="van_un")
            # Z rows land at 32-aligned partitions: tile t holds heads
            # 4t..4t+3 at partitions 0/32/64/96 (others memset to 1.0 so the
            # batched reciprocal stays finite; sel zeros mask them out).
            zbig = [ap.tile([P, RV], f32, name=f"zbig{t}") for t in range(3)]
            for t in range(3):
                nc.vector.memset(zbig[t][:], 1.0)
            vanT = ap.tile([P, KC, RV], bf16, name="vanT")

            def pair(hp):
                h0, h1 = 2 * hp, 2 * hp + 1
                hc = hp
                zt, zp = zbig[hp // 2], 64 * (hp % 2)
                e0 = lp.tile([P, JC, RV], bf16, name=f"expT{h0}", tag="expT",
                             bufs=3)
                e1_ = lp.tile([P, JC, RV], bf16, name=f"expT{h1}", tag="expT",
                              bufs=3)
                for half in range(4):
                    sc0 = psum([P, 2, RV], f"sc{h0}_{half}")
                    sc1 = psum([P, 2, RV], f"sc{h1}_{half}")
                    for jj in range(2):
                        jc = half * 2 + jj
                        # h0 on PE row groups 0-1, h1 on 2-3: adjacent issue
                        # lets the two K=64 matmuls overlap in the array.
                        nc.tensor.matmul(sc0[:, jj, :],
                                         kT[0:DH, hc, jc * P:(jc + 1) * P],
                                         qT[0:DH, hc, :],
                                         start=True, stop=True)
                        nc.tensor.matmul(sc1[:, jj, :],
                                         kT[DH:P, hc, jc * P:(jc + 1) * P],
                                         qT[DH:P, hc, :],
                                         start=True, stop=True)
                    nc.scalar.activation(e0[:, half * 2:half * 2 + 2, :],
                                         sc0[:], AF.Exp, scale=SCALE)
                    nc.scalar.activation(e1_[:, half * 2:half * 2 + 2, :],
                                         sc1[:], AF.Exp, scale=SCALE)
                pv0 = pp.tile([DH + 1, RV], f32, name=f"pv{h0}", tag="pv",
                              bufs=2)
                pv1 = pp.tile([DH + 1, RV], f32, name=f"pv{h1}", tag="pv",
                              bufs=2)
                for jc in range(JC):
                    nc.tensor.matmul(pv0[:], v_aug[:, jc, h0, :],
                                     e0[:, jc, :],
                                     start=(jc == 0), stop=(jc == JC - 1))
                    nc.tensor.matmul(pv1[:], v_aug[:, jc, h1, :],
                                     e1_[:, jc, :],
                                     start=(jc == 0), stop=(jc == JC - 1))
                nc.vector.tensor_copy(van_un[0:DH, hc, :], pv0[0:DH, :])
                nc.vector.tensor_copy(van_un[DH:P, hc, :], pv1[0:DH, :])
                nc.vector.tensor_copy(zt[zp:zp + 1, :], pv0[DH:DH + 1, :])
                nc.vector.tensor_copy(zt[zp + 32:zp + 33, :], pv1[DH:DH + 1, :])

            def normalize(t):
                # one reciprocal covers the 4 heads of zbig[t] (DVE cost
                # depends only on the free size); selector matmul broadcasts
                # the invZ rows onto channel chunks 2t, 2t+1.
                invZ = ap.tile([P, RV], f32, name=f"invZ{t}")
                nc.vector.reciprocal(invZ[:], zbig[t][:])
                invZb = ap.tile([P, RV], bf16, name=f"invZb{t}")
                nc.vector.tensor_copy(invZb[:], invZ[:])
                for i in range(2):
                    hc = 2 * t + i
                    bcp = psum([P, RV], f"bc{hc}")
                    nc.tensor.matmul(bcp[:], sel[:, i, :], invZb[:],
                                     start=True, stop=True)
                    bcs = lp.tile([P, RV], bf16, name=f"bcs{hc}", tag="bcs",
                                  bufs=2)
                    nc.vector.tensor_copy(bcs[:], bcp[:])
                    if has_vvb:
                        t0 = lp.tile([P, RV], bf16, name=f"vt{hc}", tag="vt")
                        nc.vector.tensor_mul(t0[:], van_un[:, hc, :], bcs[:])
                        nc.vector.tensor_scalar_add(vanT[:, hc, :], t0[:],
                                                    b_vv[:, hc:hc + 1])
                    else:
                        nc.vector.tensor_mul(vanT[:, hc, :], van_un[:, hc, :],
                                             bcs[:])

            # normalize(t) is emitted one pair after its inputs complete so
            # its broadcast matmuls never stall the in-order PE stream while
            # the DVE reciprocal chain catches up.
            vproj(0)
            pair(0)
            pair(1)
            for mc in range(3, KC):
                kproj(mc)
            normalize(0)
            vproj(1)
            pair(2)
            pair(3)
            pair(4)
            normalize(1)
            pair(5)
            normalize(2)

            # ---------------- gating network ---------------------------------
            def gemm(pairs, func, bias_t=None, accum_t=None, name="g",
                     out_dt=bf16, pre=None):
                out = ap.tile([P, KC, RV], out_dt, name=name)
                nmm = len(pairs) * KC
                for mc in range(KC):
                    ps = psum([P, RV], f"{name}ps{mc}")
                    i = 0
                    for wt, at in pairs:
                        for kc in range(KC):
                            nc.tensor.matmul(ps[:],
                                             wt[:, kc, mc * P:(mc + 1) * P],
                                             at[:, kc, :],
                                             start=(i == 0), stop=(i == nmm - 1))
                            i += 1
                    src = ps
                    if pre is not None:
                        tmp = lp.tile([P, RV], f32, name=f"{name}pre{mc}",
                                      tag="pretmp")
                        nc.vector.tensor_add(tmp[:], ps[:], pre[:, mc, :])
                        src = tmp
                    nc.scalar.activation(
                        out[:, mc, :], src[:], func,
                        bias=(bias_t[:, mc:mc + 1] if bias_t is not None else 0.0),
                        accum_out=(accum_t[:, mc:mc + 1]
                                   if accum_t is not None else None))
                return out

            # weights for the AR1 window fillers load ahead of time
            w_vfc = wtile("van_fc_w")
            b_vfc = btile("van_fc_b")
            w_dth1 = wtile("d_theta_w", half=1)
            w_WV = wtile("WV_w")
            w_vg0 = wtile("v_gamma_w", half=0)
            w_vo0 = wtile("van_out_w", half=0)

            theta2 = gemm([(w_vfc, vanT)], AF.Tanh, bias_t=b_vfc, name="theta2")
            e1 = gemm([(w_dth1, theta2)], AF.Exp, bias_t=bias1,
                      accum_t=parts[0], name="e1")
            co1 = remote_reduce_start(0)
            tc.no_sync_barrier()

            # --- AllReduce-1 bubble fillers (independent of z1) -------------
            gamma1 = gemm([(w_WV, vanT)], AF.Tanh, name="gamma1")
            b_vg = btile("v_gamma_b")
            z2a = gemm([(w_vg0, gamma1)], AF.Identity, bias_t=b_vg, name="z2a",
                       out_dt=f32)
            b_vo = btile("van_out_b")
            voa = gemm([(w_vo0, vanT)], AF.Identity, bias_t=b_vo, name="voa",
                       out_dt=f32)
            w_dfc = wtile("diff_fc_w")
            b_dfc = btile("diff_fc_b")
            w_vg1 = wtile("v_gamma_w", half=1)
            w_dout1 = wtile("diff_out_w", half=1)
            w_dfus = wtile("diff_fus_w")

            z1 = remote_reduce_finish(co1, "z1")
            s1 = ap.tile([P, KC], f32, name="s1")
            nc.vector.reciprocal(s1[:], z1[:])
            nc.vector.tensor_mul(s1[:], s1[:], m32[:])
            dth = ap.tile([P, KC, RV], bf16, name="dth")
            for mc in range(KC):
                nc.vector.tensor_scalar_mul(dth[:, mc, :], e1[:, mc, :],
                                            s1[:, mc:mc + 1])

            gamma2 = gemm([(w_dfc, dth)], AF.Tanh, bias_t=b_dfc, name="gamma2")

            e2 = gemm([(w_vg1, gamma2)], AF.Exp, accum_t=parts[1],
                      pre=z2a, name="e2")
            co2 = remote_reduce_start(1)
            tc.no_sync_barrier()

            # --- AllReduce-2 bubble fillers (diff branch tail) --------------
            b_dfus = btile("diff_fus_b")
            dout = gemm([(w_dout1, dth)], AF.Tanh, bias_t=bias2, name="dout")
            dfus = gemm([(w_dfus, dout)], AF.Tanh, bias_t=b_dfus, name="dfus")
            w_vo1 = wtile("van_out_w", half=1)
            w_vfus = wtile("van_fus_w")
            w_nf = wtile("nf_w")
            w_fin = wtile("final_w")

            z2 = remote_reduce_finish(co2, "z2")
            s2 = ap.tile([P, KC], f32, name="s2")
            nc.vector.reciprocal(s2[:], z2[:])
            ag = ap.tile([P, KC, RV], bf16, name="ag")
            for mc in range(KC):
                nc.vector.scalar_tensor_tensor(
                    ag[:, mc, :], e2[:, mc, :], s2[:, mc:mc + 1],
                    vanT[:, mc, :], op0=ALU.mult, op1=ALU.mult)

            vout = gemm([(w_vo1, ag)], AF.Tanh, pre=voa, name="vout")
            b_vfus = btile("van_fus_b")
            vfus = gemm([(w_vfus, vout)], AF.Tanh, bias_t=b_vfus, name="vfus")

            # gate: sigmoid(u) = 0.5*(1+tanh(u/2)); blend uses (1+tanh) bcast
            def vec_unit(wname, act_pairs, name):
                wt = wsp.tile([P, 2 * KC, 1], bf16, name=f"ws_{name}", tag="ws")
                nc.sync.dma_start(wt[:], wd[wname].rearrange(
                    "(c p) o -> p c o", p=P))
                ps = psum([1, RV], f"{name}ps")
                i = 0
                for at, base in act_pairs:
                    for kc in range(KC):
                        nc.tensor.matmul(ps[:], wt[:, base + kc, :],
                                         at[:, kc, :],
                                         start=(i == 0), stop=(i == 2 * KC - 1))
                        i += 1
                # t = tanh(u/2); tp1 = 1 + t  (so 0.5*tp1 = sigmoid(u))
                t = ap.tile([1, RV], f32, name=f"v_{name}")
                nc.scalar.activation(t[:], ps[:], AF.Tanh, scale=0.5)
                tp1 = ap.tile([1, RV], f32, name=f"vp_{name}")
                nc.vector.tensor_scalar_add(tp1[:], t[:], 1.0)
                return tp1

            gtp = vec_unit("gate_w", [(dfus, 0), (vfus, KC)], "gate")
            gbp = psum([P, RV], "gbc")
            nc.tensor.matmul(gbp[:], ones128[:], gtp[:], start=True, stop=True)
            gbs = ap.tile([P, RV], bf16, name="gbs")
            nc.vector.tensor_copy(gbs[:], gbp[:])

            # fus = dfus + 0.5*(1+t)*(vfus-dfus)
            fus = ap.tile([P, KC, RV], bf16, name="fus")
            for mc in range(KC):
                t1 = lp.tile([P, RV], bf16, name=f"ft1_{mc}", tag="ft1")
                nc.vector.tensor_sub(t1[:], vfus[:, mc, :], dfus[:, mc, :])
                t2 = lp.tile([P, RV], bf16, name=f"ft2_{mc}", tag="ft2")
                nc.vector.tensor_mul(t2[:], t1[:], gbs[:])
                nc.vector.scalar_tensor_tensor(
                    fus[:, mc, :], t2[:], 0.5, dfus[:, mc, :],
                    op0=ALU.mult, op1=ALU.add)

            b_nf = btile("nf_b")
            tnf = gemm([(w_nf, fus)], AF.Identity, bias_t=b_nf, name="tnf")
            ntp = vec_unit("nf_out_w", [(vanT, 0), (tnf, KC)], "nf")
            nbp = psum([P, RV], "nbc")
            nc.tensor.matmul(nbp[:], ones128[:], ntp[:], start=True, stop=True)
            nbs = ap.tile([P, RV], bf16, name="nbs")
            nc.vector.tensor_copy(nbs[:], nbp[:])

            b_fin = btile("final_b")
            ft = gemm([(w_fin, fus)], AF.Tanh, bias_t=b_fin, name="ftanh")
            outT = ap.tile([P, KC, RV], f32, name="outT")
            for mc in range(KC):
                # out = sigmoid(nf)*tanh(final) = 0.5*(1+t_nf)*ft
                nc.vector.scalar_tensor_tensor(
                    outT[:, mc, :], ft[:, mc, :], 0.5, nbs[:],
                    op0=ALU.mult, op1=ALU.mult)
            nc.sync.dma_start(out_d.rearrange("(mc p) n -> p mc n", p=P), outT[:])

    nc.compile()
    return nc


_CACHE = {}


def _sel_matrix():
    # sel[p, i*128+c] = 1 maps the invZ row parked at partition p onto the
    # channel half c of local chunk i: heads sit at partitions 0/32/64/96.
    m = np.zeros((P, 2 * P), np.float32)
    for i in range(2):
        m[64 * i, i * P:i * P + DH] = 1.0
        m[64 * i + 32, i * P + DH:(i + 1) * P] = 1.0
    return np.ascontiguousarray(m.astype(nbf16))


def kernel(**inputs):
    x = np.asarray(inputs["x"], np.float32)
    y = np.asarray(inputs["y"], np.float32)
    has_vvb = bool(np.any(np.asarray(inputs["vv_b"]) != 0))
    if has_vvb not in _CACHE:
        _CACHE[has_vvb] = build(has_vvb)
    nc = _CACHE[has_vvb]

    xt = np.ascontiguousarray(x.reshape(B * S, H).T).astype(nbf16)   # [H, 2048]
    yts = [np.ascontiguousarray(y[b].T).astype(nbf16) for b in range(B)]

    base = {}
    for w in W768 + W1536 + ["gate_w", "nf_out_w"]:
        base[w] = np.asarray(inputs[w], np.float32).astype(nbf16)
    for b in BIAS:
        base[b] = np.ascontiguousarray(np.asarray(inputs[b], np.float32))
    if has_vvb:
        base["vv_b"] = np.ascontiguousarray(np.asarray(inputs["vv_b"], np.float32))
    base["selM"] = _sel_matrix()

    in_maps = []
    for c in range(8):
        bat = c // 4
        m = dict(base)
        m["xT"] = np.ascontiguousarray(xt[:, c * RV:(c + 1) * RV])
        m["yT"] = yts[bat]
        in_maps.append(m)

    res = run_bass_kernel_spmd(nc, in_maps, core_ids=list(range(8)))
    full = np.concatenate([res.results[c]["outT"] for c in range(8)], axis=1)
    return np.ascontiguousarray(full.T.reshape(B, S, H)).astype(np.float32)


if __name__ == "__main__":
    rng = np.random.default_rng(0)
    ins = {"x": rng.standard_normal((B, S, H)).astype(np.float32),
           "y": rng.standard_normal((B, S, H)).astype(np.float32)}
    for w in W768 + W1536:
        shp = (H, H) if w in W768 else (2 * H, H)
        ins[w] = (rng.standard_normal(shp) * 0.02).astype(np.float32)
    ins["gate_w"] = (rng.standard_normal((2 * H, 1)) * 0.02).astype(np.float32)
    ins["nf_out_w"] = (rng.standard_normal((2 * H, 1)) * 0.02).astype(np.float32)
    for b in BIAS + ["vv_b"]:
        ins[b] = np.zeros(H, np.float32)
    out = kernel(**ins)
    print("out", out.shape, out.dtype, np.abs(out).mean())


# revision 42
# speedup vs baseline: 1.0508x; 1.0508x over previous
"""Coupled-attention module as a distributed Bass/Tile kernel on 8 TRN2 cores.

Math notes (exact algebra, not approximations):
- The differential-attention scores are constant along the softmax axis, so
  softmax yields exactly uniform 1/S weights: diff_vector collapses to the
  per-batch mean of (y @ dv_w + dv_b), broadcast over sequence. dq/dk are dead.
- Sharding: rows of the flattened (B*S, H) activations, 256 per core; cores
  0-3 own batch 0, 4-7 batch 1. Each core redundantly computes full-batch K/V
  (cheaper than any reshard collective at this scale).
- All activations live channel-major [C, rows] on chip, so weights feed the
  PE as natural [K, M] lhsT tiles, and the two sequence-axis softmaxes in the
  gating network reduce along the free dim. Their denominators are summed
  across the 4-core batch group with tiny AllReduces.
- Compute in bf16 with fp32 accumulation (all GEMMs), exp/tanh on ACT.
- Attention softmax normalization is deferred: per head the unnormalized
  PV and the Z row are copied out, then ONE batched reciprocal [12, 256]
  plus a selector-matmul broadcast rescales all heads (the per-head DVE
  reciprocal at [1, 256] costs the same as the whole batch).
- Sigmoids are computed as 0.5*(1+tanh(x/2)) to stay in the exp/tanh ACT
  table set (avoids a ~1.5us ACT table switch).
- no_sync_barrier after each AllReduce input keeps the filler GEMMs from
  being consumed early, so they land inside the collective's window.
"""

import numpy as np
import ml_dtypes

import concourse.bass as bass
import concourse.mybir as mybir
import concourse.tile as tile
from concourse import bacc
from concourse.bass_utils import run_bass_kernel_spmd

B, S, H = 2, 1024, 768
NH, DH = 12, 64
P = 128
RV = 256            # rows per core
KC = H // P         # 6 channel chunks
JC = S // P         # 8 sequence chunks
GROUPS = [[0, 1, 2, 3], [4, 5, 6, 7]]
SCALE = 1.0 / 8.0   # 1/sqrt(DH)

bf16 = mybir.dt.bfloat16
f32 = mybir.dt.float32
AF = mybir.ActivationFunctionType
ALU = mybir.AluOpType
nbf16 = ml_dtypes.bfloat16

W768 = ["vq_w", "vk_w", "vv_w", "dv_w", "WD_w", "van_fc_w", "WV_w", "diff_fc_w",
        "diff_fus_w", "van_fus_w", "nf_w", "final_w"]
W1536 = ["d_theta_w", "v_gamma_w", "diff_out_w", "van_out_w"]
BIAS = ["vq_b", "vk_b", "dv_b", "van_fc_b", "d_theta_b", "diff_fc_b",
        "v_gamma_b", "diff_out_b", "van_out_b", "diff_fus_b", "van_fus_b",
        "nf_b", "final_b"]


def build(has_vvb: bool):
    nc = bacc.Bacc(None, target_bir_lowering=False, debug=False, num_devices=8)

    xT_d = nc.dram_tensor("xT", [H, RV], bf16, kind="ExternalInput")
    yT_d = nc.dram_tensor("yT", [H, S], bf16, kind="ExternalInput")
    wd = {}
    for w in W768:
        wd[w] = nc.dram_tensor(w, [H, H], bf16, kind="ExternalInput")
    for w in W1536:
        wd[w] = nc.dram_tensor(w, [2 * H, H], bf16, kind="ExternalInput")
    wd["gate_w"] = nc.dram_tensor("gate_w", [2 * H, 1], bf16, kind="ExternalInput")
    wd["nf_out_w"] = nc.dram_tensor("nf_out_w", [2 * H, 1], bf16, kind="ExternalInput")
    bd = {}
    for b in BIAS:
        bd[b] = nc.dram_tensor(b, [H], f32, kind="ExternalInput")
    if has_vvb:
        bd["vv_b"] = nc.dram_tensor("vv_b", [H], f32, kind="ExternalInput")
    sel_d = nc.dram_tensor("selM", [P, 2 * P], bf16, kind="ExternalInput")
    out_d = nc.dram_tensor("outT", [H, RV], f32, kind="ExternalOutput")

    with tile.TileContext(nc, num_cores=8) as tc:
        with (
            tc.tile_pool(name="wpool", bufs=6) as wp,
            tc.tile_pool(name="wsmall", bufs=2) as wsp,
            tc.tile_pool(name="acts", bufs=1) as ap,
            tc.tile_pool(name="loop", bufs=2) as lp,
            tc.tile_pool(name="psum", bufs=8, space="PSUM") as pp,
            tc.tile_pool(name="dram", bufs=4, space="DRAM") as dp,
        ):
            # PSUM budget: tag "ps" = 6 x 1-bank [128, 512] f32 slots shared
            # by every projection/score/gating matmul; tag "pv" = 2 x 1-bank
            # [65, 2, 256] accumulators. 6 + 2 = 8 banks exactly.
            def psum(shape, name):
                return pp.tile(shape, f32, name=name, tag="ps", bufs=6)

            # -------- group reduce via ncfw AllReduce. The accum ACTs write
            # the partial into part[r]; the reduced value is read back from
            # DRAM. A dummy warmup AllReduce early in the kernel absorbs the
            # ~12us first-collective ncfw spin-up observed on every trace.
            parts = [ap.tile([P, KC], f32, name=f"part{r}") for r in range(2)]

            def remote_reduce_start(r):
                ci = dp.tile([P, KC], f32, name=f"ci{r}")
                co = dp.tile([P, KC], f32, name=f"co{r}")
                nc.sync.dma_start(ci[:], parts[r][:])
                nc.gpsimd.collective_compute(
                    "AllReduce", ALU.add, replica_groups=GROUPS,
                    ins=[ci[:]], outs=[co[:]])
                return co

            def remote_reduce_finish(co, name):
                z = ap.tile([P, KC], f32, name=f"zz_{name}")
                nc.sync.dma_start(z[:], co[:])
                return z

            def wtile(name, half=None):
                t = wp.tile([P, KC, H], bf16, name=f"w_{name}_{half}", tag="w")
                src = wd[name]
                if half is not None:
                    src = src[half * H:(half + 1) * H, :]
                src = src.rearrange("(kc p) n -> kc p n", p=P)
                for kc in range(KC):
                    nc.sync.dma_start(t[:, kc, :], src[kc])
                return t

            def btile(name):
                t = ap.tile([P, KC], f32, name=f"b_{name}")
                nc.sync.dma_start(t[:], bd[name].rearrange("(c p) -> p c", p=P))
                return t

            # K-side inputs stream first: K proj is the startup long pole.
            b_vk = btile("vk_b")
            b_dv = btile("dv_b")
            yT = ap.tile([P, KC, S], bf16, name="yT")
            for kc in range(KC):
                nc.sync.dma_start(yT[:, kc, :], yT_d.rearrange(
                    "(kc p) n -> kc p n", p=P)[kc])
            w_vk0 = wtile("vk_w")

            b_vq = btile("vq_b")
            xT = ap.tile([P, KC, RV], bf16, name="xT")
            for kc in range(KC):
                nc.sync.dma_start(xT[:, kc, :], xT_d.rearrange(
                    "(kc p) n -> kc p n", p=P)[kc])
            w_vq = wtile("vq_w")
            qT = ap.tile([P, KC, RV], bf16, name="qT")
            for mc in range(KC):
                ps = psum([P, RV], f"qps{mc}")
                for kc in range(KC):
                    nc.tensor.matmul(ps[:], w_vq[:, kc, mc * P:(mc + 1) * P],
                                     xT[:, kc, :],
                                     start=(kc == 0), stop=(kc == KC - 1))
                nc.scalar.activation(qT[:, mc, :], ps[:], AF.Identity,
                                     bias=b_vq[:, mc:mc + 1])

            ones128 = ap.tile([1, P], f32, name="ones128")
            nc.vector.memset(ones128[:], 1.0)

            # dummy warmup collective: absorbs the ~12us ncfw first-op
            # spin-up while the PE is busy with projections
            wu = ap.tile([1, 1], f32, name="wu")
            nc.vector.memset(wu[:], 0.0)
            wci = dp.tile([1, 1], f32, name="wci")
            wco = dp.tile([1, 1], f32, name="wco")
            nc.sync.dma_start(wci[:], wu[:])
            nc.gpsimd.collective_compute(
                "AllReduce", ALU.add, replica_groups=GROUPS,
                ins=[wci[:]], outs=[wco[:]])

            # selector for broadcasting invZ rows (at partitions 0/32/64/96)
            # onto the two channel chunks each zbig tile covers
            sel = ap.tile([P, 2, P], bf16, name="sel")
            nc.sync.dma_start(sel[:], sel_d.rearrange("p (c q) -> p c q", q=P))

            # ---------------- K / V projections (emitted interleaved with
            # the attention pairs below so PE stays dense through the
            # ACT-bound exp phases; HAM stays warm) -----------------------
            w_vk = w_vk0
            kT = ap.tile([P, KC, S], bf16, name="kT")

            def kproj(mc):
                for nh in range(2):
                    ps = psum([P, 512], f"kps{mc}_{nh}")
                    for kc in range(KC):
                        nc.tensor.matmul(
                            ps[:], w_vk[:, kc, mc * P:(mc + 1) * P],
                            yT[:, kc, nh * 512:(nh + 1) * 512],
                            start=(kc == 0), stop=(kc == KC - 1))
                    nc.scalar.activation(kT[:, mc, nh * 512:(nh + 1) * 512], ps[:],
                                         AF.Identity, bias=b_vk[:, mc:mc + 1])

            w_vv = wtile("vv_w")
            v_aug = ap.tile([P, JC, NH, DH + 1], bf16, name="v_aug")
            nc.vector.memset(v_aug[:, :, :, DH:DH + 1], 1.0)

            def vproj(cg):
                for jc in range(JC):
                    ps = psum([P, 384], f"vps{jc}_{cg}")
                    for kc in range(KC):
                        nc.tensor.matmul(
                            ps[:], yT[:, kc, jc * P:(jc + 1) * P],
                            w_vv[:, kc, cg * 384:(cg + 1) * 384],
                            start=(kc == 0), stop=(kc == KC - 1))
                    nc.vector.tensor_copy(
                        v_aug[:, jc, cg * 6:(cg + 1) * 6, 0:DH],
                        ps[:].rearrange("p (h d) -> p h d", d=DH))

            for mc in range(3):
                kproj(mc)

            # ---------------- diff-branch constants (per batch) -------------
            # m = mean_s(y) @ dv_w + dv_b ; theta1 = tanh(m @ WD_w)
            # bias1 = theta1 @ d_theta_w[:H] + d_theta_b
            # bias2 = m @ diff_out_w[:H] + diff_out_b
            yb = ap.tile([P, KC], f32, name="yb")
            ybt = ap.tile([P, KC], bf16, name="ybt")
            for kc in range(KC):
                nc.vector.tensor_reduce(yb[:, kc:kc + 1], yT[:, kc, :],
                                        axis=mybir.AxisListType.X, op=ALU.add)
            nc.vector.tensor_scalar_mul(ybt[:], yb[:], 1.0 / S)

            def vec_chain(w_t, rhs_t, func, bias_t, out_dt, name):
                out = ap.tile([P, KC], out_dt, name=name)
                for mc in range(KC):
                    ps = psum([P, 1], f"{name}ps{mc}")
                    for kc in range(KC):
                        nc.tensor.matmul(ps[:], w_t[:, kc, mc * P:(mc + 1) * P],
                                         rhs_t[:, kc:kc + 1],
                                         start=(kc == 0), stop=(kc == KC - 1))
                    nc.scalar.activation(out[:, mc:mc + 1], ps[:], func,
                                         bias=(bias_t[:, mc:mc + 1]
                                               if bias_t is not None else 0.0))
                return out

            w_dv = wtile("dv_w")
            m32 = vec_chain(w_dv, ybt, AF.Identity, b_dv, f32, "m32")
            mbf = ap.tile([P, KC], bf16, name="mbf")
            nc.vector.tensor_copy(mbf[:], m32[:])
            w_WD = wtile("WD_w")
            th1 = vec_chain(w_WD, mbf, AF.Tanh, None, bf16, "th1")
            w_dth0 = wtile("d_theta_w", half=0)
            b_dth = btile("d_theta_b")
            bias1 = vec_chain(w_dth0, th1, AF.Identity, b_dth, f32, "bias1")
            w_dout0 = wtile("diff_out_w", half=0)
            b_dout = btile("diff_out_b")
            bias2 = vec_chain(w_dout0, mbf, AF.Identity, b_dout, f32, "bias2")

            # ---------------- attention (12 heads, 256 own queries) ---------
            # Unnormalized PV + Z rows per head; normalization batched in two
            # head groups (0-7, 8-11) so vanT chunks 0-3 free up early.
            if has_vvb:
                b_vv = btile("vv_b")
            van_un = ap.tile([P, KC, RV], bf16, name="van_un")
            # Z rows land at 32-aligned partitions: tile t holds heads
            # 4t..4t+3 at partitions 0/32/64/96 (others memset to 1.0 so the
            # batched reciprocal stays finite; sel zeros mask them out).
            zbig = [ap.tile([P, RV], f32, name=f"zbig{t}") for t in range(3)]
            for t in range(3):
                nc.vector.memset(zbig[t][:], 1.0)
            vanT = ap.tile([P, KC, RV], bf16, name="vanT")

            def pair(hp):
                h0, h1 = 2 * hp, 2 * hp + 1
                hc = hp
                zt, zp = zbig[hp // 2], 64 * (hp % 2)
                e0 = lp.tile([P, JC, RV], bf16, name=f"expT{h0}", tag="expT",
                             bufs=3)
                e1_ = lp.tile([P, JC, RV], bf16, name=f"expT{h1}", tag="expT",
                              bufs=3)
                for half in range(4):
                    sc0 = psum([P, 2, RV], f"sc{h0}_{half}")
                    sc1 = psum([P, 2, RV], f"sc{h1}_{half}")
                    for jj in range(2):
                        jc = half * 2 + jj
                        # h0 on PE row groups 0-1, h1 on 2-3: adjacent issue
                        # lets the two K=64 matmuls overlap in the array.
                        nc.tensor.matmul(sc0[:, jj, :],
                                         kT[0:DH, hc, jc * P:(jc + 1) * P],
                                         qT[0:DH, hc, :],
                                         start=True, stop=True)
                        nc.tensor.matmul(sc1[:, jj, :],
                                         kT[DH:P, hc, jc * P:(jc + 1) * P],
                                         qT[DH:P, hc, :],
                                         start=True, stop=True)
                    nc.scalar.activation(e0[:, half * 2:half * 2 + 2, :],
                                         sc0[:], AF.Exp, scale=SCALE)
                    nc.scalar.activation(e1_[:, half * 2:half * 2 + 2, :],
                                         sc1[:], AF.Exp, scale=SCALE)
                pv0 = pp.tile([DH + 1, RV], f32, name=f"pv{h0}", tag="pv",
                              bufs=2)
                pv1 = pp.tile([DH + 1, RV], f32, name=f"pv{h1}", tag="pv",
                              bufs=2)
                for jc in range(JC):
                    nc.tensor.matmul(pv0[:], v_aug[:, jc, h0, :],
                                     e0[:, jc, :],
                                     start=(jc == 0), stop=(jc == JC - 1))
                    nc.tensor.matmul(pv1[:], v_aug[:, jc, h1, :],
                                     e1_[:, jc, :],
                                     start=(jc == 0), stop=(jc == JC - 1))
                nc.vector.tensor_copy(van_un[0:DH, hc, :], pv0[0:DH, :])
                nc.vector.tensor_copy(van_un[DH:P, hc, :], pv1[0:DH, :])
                nc.vector.tensor_copy(zt[zp:zp + 1, :], pv0[DH:DH + 1, :])
                nc.vector.tensor_copy(zt[zp + 32:zp + 33, :], pv1[DH:DH + 1, :])

            def normalize(t):
                # one reciprocal covers the 4 heads of zbig[t] (DVE cost
                # depends only on the free size); selector matmul broadcasts
                # the invZ rows onto channel chunks 2t, 2t+1.
                invZ = ap.tile([P, RV], f32, name=f"invZ{t}")
                nc.vector.reciprocal(invZ[:], zbig[t][:])
                invZb = ap.tile([P, RV], bf16, name=f"invZb{t}")
                nc.vector.tensor_copy(invZb[:], invZ[:])
                for i in range(2):
                    hc = 2 * t + i
                    bcp = psum([P, RV], f"bc{hc}")
                    nc.tensor.matmul(bcp[:], sel[:, i, :], invZb[:],
                                     start=True, stop=True)
                    bcs = lp.tile([P, RV], bf16, name=f"bcs{hc}", tag="bcs",
                                  bufs=2)
                    nc.vector.tensor_copy(bcs[:], bcp[:])
                    if has_vvb:
                        t0 = lp.tile([P, RV], bf16, name=f"vt{hc}", tag="vt")
                        nc.vector.tensor_mul(t0[:], van_un[:, hc, :], bcs[:])
                        nc.vector.tensor_scalar_add(vanT[:, hc, :], t0[:],
                                                    b_vv[:, hc:hc + 1])
                    else:
                        nc.vector.tensor_mul(vanT[:, hc, :], van_un[:, hc, :],
                                             bcs[:])

            # normalize(t) is emitted one pair after its inputs complete so
            # its broadcast matmuls never stall the in-order PE stream while
            # the DVE reciprocal chain catches up.
            vproj(0)
            pair(0)
            pair(1)
            for mc in range(3, KC):
                kproj(mc)
            normalize(0)
            vproj(1)
            pair(2)
            pair(3)
            pair(4)
            normalize(1)
            pair(5)
            normalize(2)

            # ---------------- gating network ---------------------------------
            def gemm(pairs, func, bias_t=None, accum_t=None, name="g",
                     out_dt=bf16, pre=None):
                out = ap.tile([P, KC, RV], out_dt, name=name)
                nmm = len(pairs) * KC
                for mc in range(KC):
                    ps = psum([P, RV], f"{name}ps{mc}")
                    i = 0
                    for wt, at in pairs:
                        for kc in range(KC):
                            nc.tensor.matmul(ps[:],
                                             wt[:, kc, mc * P:(mc + 1) * P],
                                             at[:, kc, :],
                                             start=(i == 0), stop=(i == nmm - 1))
                            i += 1
                    src = ps
                    if pre is not None:
                        tmp = lp.tile([P, RV], f32, name=f"{name}pre{mc}",
                                      tag="pretmp")
                        nc.vector.tensor_add(tmp[:], ps[:], pre[:, mc, :])
                        src = tmp
                    nc.scalar.activation(
                        out[:, mc, :], src[:], func,
                        bias=(bias_t[:, mc:mc + 1] if bias_t is not None else 0.0),
                        accum_out=(accum_t[:, mc:mc + 1]
                                   if accum_t is not None else None))
                return out

            # weights for the AR1 window fillers load ahead of time
            w_vfc = wtile("van_fc_w")
            b_vfc = btile("van_fc_b")
            w_dth1 = wtile("d_theta_w", half=1)
            w_WV = wtile("WV_w")
            w_vg0 = wtile("v_gamma_w", half=0)
            w_vo0 = wtile("van_out_w", half=0)

            theta2 = gemm([(w_vfc, vanT)], AF.Tanh, bias_t=b_vfc, name="theta2")
            e1 = gemm([(w_dth1, theta2)], AF.Exp, bias_t=bias1,
                      accum_t=parts[0], name="e1")
            co1 = remote_reduce_start(0)
            tc.no_sync_barrier()

            # --- AllReduce-1 bubble fillers (independent of z1) -------------
            gamma1 = gemm([(w_WV, vanT)], AF.Tanh, name="gamma1")
            b_vg = btile("v_gamma_b")
            z2a = gemm([(w_vg0, gamma1)], AF.Identity, bias_t=b_vg, name="z2a",
                       out_dt=f32)
            b_vo = btile("van_out_b")
            voa = gemm([(w_vo0, vanT)], AF.Identity, bias_t=b_vo, name="voa",
                       out_dt=f32)
            w_dfc = wtile("diff_fc_w")
            b_dfc = btile("diff_fc_b")
            w_vg1 = wtile("v_gamma_w", half=1)
            w_dout1 = wtile("diff_out_w", half=1)
            w_dfus = wtile("diff_fus_w")

            z1 = remote_reduce_finish(co1, "z1")
            s1 = ap.tile([P, KC], f32, name="s1")
            nc.vector.reciprocal(s1[:], z1[:])
            nc.vector.tensor_mul(s1[:], s1[:], m32[:])
            dth = ap.tile([P, KC, RV], bf16, name="dth")
            for mc in range(KC):
                nc.vector.tensor_scalar_mul(dth[:, mc, :], e1[:, mc, :],
                                            s1[:, mc:mc + 1])

            gamma2 = gemm([(w_dfc, dth)], AF.Tanh, bias_t=b_dfc, name="gamma2")

            e2 = gemm([(w_vg1, gamma2)], AF.Exp, accum_t=parts[1],
                      pre=z2a, name="e2")
            co2 = remote_reduce_start(1)
            tc.no_sync_barrier()

            # --- AllReduce-2 bubble fillers (diff branch tail) --------------
            b_dfus = btile("diff_fus_b")
            dout = gemm([(w_dout1, dth)], AF.Tanh, bias_t=bias2, name="dout")
            dfus = gemm([(w_dfus, dout)], AF.Tanh, bias_t=b_dfus, name="dfus")
            w_vo1 = wtile("van_out_w", half=1)
            w_vfus = wtile("van_fus_w")
            w_nf = wtile("nf_w")
            w_fin = wtile("final_w")

            # PE warm-keeper for the rest of the AR2 window: ~6us of matmuls
            # on resident tiles (results discarded) so the HAM clock stays at
            # 8/8 and the post-collective tail starts at 2.4 GHz, not 1.2.
            # Sized below the smallest observed AR2 window so it never
            # delays the tail. The 1-element copies defeat dead-code elim.
            wsink = ap.tile([1, 16], f32, name="wsink")
            for i in range(9):
                wps = psum([P, RV], f"warm{i}")
                for j in range(KC):
                    nc.tensor.matmul(wps[:], vanT[:, (i + j) % KC, 0:P],
                                     vanT[:, j, :],
                                     start=(j == 0), stop=(j == KC - 1))
                nc.vector.tensor_copy(wsink[0:1, i:i + 1], wps[0:1, 0:1])

            z2 = remote_reduce_finish(co2, "z2")
            s2 = ap.tile([P, KC], f32, name="s2")
            nc.vector.reciprocal(s2[:], z2[:])
            ag = ap.tile([P, KC, RV], bf16, name="ag")
            for mc in range(KC):
                nc.vector.scalar_tensor_tensor(
                    ag[:, mc, :], e2[:, mc, :], s2[:, mc:mc + 1],
                    vanT[:, mc, :], op0=ALU.mult, op1=ALU.mult)

            vout = gemm([(w_vo1, ag)], AF.Tanh, pre=voa, name="vout")
            b_vfus = btile("van_fus_b")
            vfus = gemm([(w_vfus, vout)], AF.Tanh, bias_t=b_vfus, name="vfus")

            # gate: sigmoid(u) = 0.5*(1+tanh(u/2)); blend uses (1+tanh) bcast
            def vec_unit(wname, act_pairs, name):
                wt = wsp.tile([P, 2 * KC, 1], bf16, name=f"ws_{name}", tag="ws")
                nc.sync.dma_start(wt[:], wd[wname].rearrange(
                    "(c p) o -> p c o", p=P))
                ps = psum([1, RV], f"{name}ps")
                i = 0
                for at, base in act_pairs:
                    for kc in range(KC):
                        nc.tensor.matmul(ps[:], wt[:, base + kc, :],
                                         at[:, kc, :],
                                         start=(i == 0), stop=(i == 2 * KC - 1))
                        i += 1
                # t = tanh(u/2); tp1 = 1 + t  (so 0.5*tp1 = sigmoid(u))
                t = ap.tile([1, RV], f32, name=f"v_{name}")
                nc.scalar.activation(t[:], ps[:], AF.Tanh, scale=0.5)
                tp1 = ap.tile([1, RV], f32, name=f"vp_{name}")
                nc.vector.tensor_scalar_add(tp1[:], t[:], 1.0)
                return tp1

            gtp = vec_unit("gate_w", [(dfus, 0), (vfus, KC)], "gate")
            gbp = psum([P, RV], "gbc")
            nc.tensor.matmul(gbp[:], ones128[:], gtp[:], start=True, stop=True)
            gbs = ap.tile([P, RV], bf16, name="gbs")
            nc.vector.tensor_copy(gbs[:], gbp[:])

            # fus = dfus + 0.5*(1+t)*(vfus-dfus)
            fus = ap.tile([P, KC, RV], bf16, name="fus")
            for mc in range(KC):
                t1 = lp.tile([P, RV], bf16, name=f"ft1_{mc}", tag="ft1")
                nc.vector.tensor_sub(t1[:], vfus[:, mc, :], dfus[:, mc, :])
                t2 = lp.tile([P, RV], bf16, name=f"ft2_{mc}", tag="ft2")
                nc.vector.tensor_mul(t2[:], t1[:], gbs[:])
                nc.vector.scalar_tensor_tensor(
                    fus[:, mc, :], t2[:], 0.5, dfus[:, mc, :],
                    op0=ALU.mult, op1=ALU.add)

            b_nf = btile("nf_b")
            tnf = gemm([(w_nf, fus)], AF.Identity, bias_t=b_nf, name="tnf")
            ntp = vec_unit("nf_out_w", [(vanT, 0), (tnf, KC)], "nf")
            nbp = psum([P, RV], "nbc")
            nc.tensor.matmul(nbp[:], ones128[:], ntp[:], start=True, stop=True)
            nbs = ap.tile([P, RV], bf16, name="nbs")
            nc.vector.tensor_copy(nbs[:], nbp[:])

            b_fin = btile("final_b")
            ft = gemm([(w_fin, fus)], AF.Tanh, bias_t=b_fin, name="ftanh")
            outT = ap.tile([P, KC, RV], f32, name="outT")
            for mc in range(KC):
                # out = sigmoid(nf)*tanh(final) = 0.5*(1+t_nf)*ft
                nc.vector.scalar_tensor_tensor(
                    outT[:, mc, :], ft[:, mc, :], 0.5, nbs[:],
                    op0=ALU.mult, op1=ALU.mult)
            nc.sync.dma_start(out_d.rearrange("(mc p) n -> p mc n", p=P), outT[:])

    nc.compile()
    return nc


_CACHE = {}


def _sel_matrix():
    # sel[p, i*128+c] = 1 maps the invZ row parked at partition p onto the
    # channel half c of local chunk i: heads sit at partitions 0/32/64/96.
    m = np.zeros((P, 2 * P), np.float32)
    for i in range(2):
        m[64 * i, i * P:i * P + DH] = 1.0
        m[64 * i + 32, i * P + DH:(i + 1) * P] = 1.0
    return np.ascontiguousarray(m.astype(nbf16))


def kernel(**inputs):
    x = np.asarray(inputs["x"], np.float32)
    y = np.asarray(inputs["y"], np.float32)
    has_vvb = bool(np.any(np.asarray(inputs["vv_b"]) != 0))
    if has_vvb not in _CACHE:
        _CACHE[has_vvb] = build(has_vvb)
    nc = _CACHE[has_vvb]

    xt = np.ascontiguousarray(x.reshape(B * S, H).T).astype(nbf16)   # [H, 2048]
    yts = [np.ascontiguousarray(y[b].T).astype(nbf16) for b in range(B)]

    base = {}
    for w in W768 + W1536 + ["gate_w", "nf_out_w"]:
        base[w] = np.asarray(inputs[w], np.float32).astype(nbf16)
    for b in BIAS:
        base[b] = np.ascontiguousarray(np.asarray(inputs[b], np.float32))
    if has_vvb:
        base["vv_b"] = np.ascontiguousarray(np.asarray(inputs["vv_b"], np.float32))
    base["selM"] = _sel_matrix()

    in_maps = []
    for c in range(8):
        bat = c // 4
        m = dict(base)
        m["xT"] = np.ascontiguousarray(xt[:, c * RV:(c + 1) * RV])
        m["yT"] = yts[bat]
        in_maps.append(m)

    res = run_bass_kernel_spmd(nc, in_maps, core_ids=list(range(8)))
    full = np.concatenate([res.results[c]["outT"] for c in range(8)], axis=1)
    return np.ascontiguousarray(full.T.reshape(B, S, H)).astype(np.float32)


if __name__ == "__main__":
    rng = np.random.default_rng(0)
    ins = {"x": rng.standard_normal((B, S, H)).astype(np.float32),
           "y": rng.standard_normal((B, S, H)).astype(np.float32)}
    for w in W768 + W1536:
        shp = (H, H) if w in W768 else (2 * H, H)
        ins[w] = (rng.standard_normal(shp) * 0.02).astype(np.float32)
    ins["gate_w"] = (rng.standard_normal((2 * H, 1)) * 0.02).astype(np.float32)
    ins["nf_out_w"] = (rng.standard_normal((2 * H, 1)) * 0.02).astype(np.float32)
    for b in BIAS + ["vv_b"]:
        ins[b] = np.zeros(H, np.float32)
    out = kernel(**ins)
    print("out", out.shape, out.dtype, np.abs(out).mean())
